# revision 10
# baseline (speedup 1.0000x reference)
"""Trainium2 Bass kernel for nn_BiasFreeClifordNetBlock.

Self-contained: takes FULL inputs (as from setup_inputs()), shards across 8
NeuronCores (batch x H-half data parallel), runs one SPMD Bass program, and
reassembles the full output.

Per-core layout: channel-major compute ([C=2x128 partitions, tokens free]).
  - LayerNorm stats via bn_stats (token-major), normalize+cast to bf16,
    PE-transpose to channel-major with ln_gamma folded into the PSUM->SBUF evac.
  - Two 3x3 depthwise convs as 9 fused scalar_tensor_tensor MAC taps each over
    a zero-padded spatial slab; BN+SiLU folded into one scalar-engine op.
  - Sparse rolling geometric products reformulated as 10 channel-diagonal
    products (channel rolls materialized by tiny SBUF->SBUF DMAs); the
    inner/wedge +/- structure and the global-mean branch are folded into
    host-side weight transforms (see algebra_check.py):
      g = sum_s R_s @ (Wi+Ww) + T_s @ (Wi-Ww)  (local, s in 1,2,4,8)
        + sum_s S_s @ (-2*Wi_glo)              (global self part, s in 1,2)
        + z_det @ G_img                        (global mean part)
    where R_s = zd*roll(b,s), T_s = roll(zd,s)*b, S_s = zd*roll(zd,s), and
    G_img is a per-image 256x256 matrix built on-device from the image mean m.
  - m is computed without cross-core collectives: own-half sums ride the
    evacuation accum_out; other-half sums use bn_stats + tiny PE matmuls
    against the re-streamed raw x.
All matmuls in bf16 (fp32 accumulation in PSUM). The final out = x + h_mix is
fp32, and h_mix is scaled by gamma_ls (=1e-5), so bf16 intermediate error is
damped ~5 orders of magnitude below the output norm.
"""
import numpy as np
import ml_dtypes

import concourse.bacc as bacc
import concourse.tile as tile
from concourse import mybir
from concourse import bass_utils

F32 = mybir.dt.float32
BF16 = mybir.dt.bfloat16
AL = mybir.AluOpType
AF = mybir.ActivationFunctionType
AX = mybir.AxisListType
BFNP = ml_dtypes.bfloat16

B, H, W, C = 4, 128, 128, 256
N_CORES = 8
HS = H // 2              # 64 rows per core
LN_EPS, BN_EPS = 1e-6, 1e-3
SHIFTS_L = (1, 2, 4, 8)
SHIFTS_G = (1, 2)
NG = HS // 4             # 16 own groups of 4 rows
SGTOK = 512              # tokens per product supergroup
NSG = HS * W // SGTOK    # 8 supergroups
# phase-B K-block order (must match host wgeo packing)
BLOCKS = [("R", 1), ("T", 1), ("S", 1), ("R", 2), ("T", 2), ("S", 2),
          ("R", 4), ("T", 4), ("R", 8), ("T", 8)]


def _roll_dma(nc, dst, src_pair, s, fwin):
    """dst[kc][c] = src[(c - s) mod 256] over free window fwin=(lo, hi).

    dst: tile [128, 2, N]; src_pair: (ap0, ap1) full-slab APs per chunk whose
    free dim is sliced by fwin. 4 DMAs (partition-wrap split)."""
    lo, hi = fwin
    s0, s1 = src_pair
    nc.sync.dma_start(out=dst[s:128, 0, :], in_=s0[0:128 - s, lo:hi])
    nc.sync.dma_start(out=dst[0:s, 0, :], in_=s1[128 - s:128, lo:hi])
    nc.sync.dma_start(out=dst[s:128, 1, :], in_=s1[0:128 - s, lo:hi])
    nc.sync.dma_start(out=dst[0:s, 1, :], in_=s0[128 - s:128, lo:hi])


def _mroll_dma(nc, dst, m_t, s, col):
    """dst[:, col+kc] = rolled m: out global row r gets m[(r - s) mod 256].
    m_t is [128, 2] (chunk columns); s may be negative."""
    for kc in (0, 1):
        other = 1 - kc
        if s > 0:
            nc.sync.dma_start(out=dst[s:128, col + kc:col + kc + 1],
                              in_=m_t[0:128 - s, kc:kc + 1])
            nc.sync.dma_start(out=dst[0:s, col + kc:col + kc + 1],
                              in_=m_t[128 - s:128, other:other + 1])
        else:
            t = -s
            nc.sync.dma_start(out=dst[0:128 - t, col + kc:col + kc + 1],
                              in_=m_t[t:128, kc:kc + 1])
            nc.sync.dma_start(out=dst[128 - t:128, col + kc:col + kc + 1],
                              in_=m_t[0:t, other:other + 1])


def build_program(n_cores=N_CORES):
    nc = bacc.Bacc("TRN2", target_bir_lowering=False, debug=False,
                   enable_asserts=True, num_devices=n_cores)

    def din(name, shape, dt):
        return nc.dram_tensor(name, list(shape), dt, kind="ExternalInput").ap()

    x_slab = din("x_slab", (68, W, C), F32)      # rows 0..63 own, 64..67 halo
    x_oth = din("x_oth", (HS, W, C), F32)        # other half rows (for mean)
    wdet = din("wdet", (2, 128, C), BF16)
    wgeo = din("wgeo", (20, 128, C), BF16)       # BLOCKS order, 2 kc each
    wgimg = din("wgimg", (8, 128, C), BF16)      # [u1,u2,vr1,vr2] x kc
    wgate = din("wgate", (4, 128, C), BF16)
    k1d = din("k1", (2, 128, 9), F32)
    k2d = din("k2", (2, 128, 9), F32)
    bnsb = din("bnsb", (2, 128, 2), F32)         # [:,:,0]=scale, 1=bias
    lng = din("lng", (2, 128, 1), F32)
    glsd = din("gls", (2, 128, 1), F32)
    emask = din("emask", (128, 2), F32)          # z1 edge-row masks
    y = nc.dram_tensor("y", [HS, W, C], F32, kind="ExternalOutput").ap()

    WP2, WW2 = 66, 130                            # z1 slab spatial dims
    SG = SGTOK // 512                             # groups per supergroup

    with tile.TileContext(nc) as tc:
        with tc.tile_pool(name="main", bufs=1) as main:
            # ---------- persistent tiles ----------
            xnT = [main.tile([128, 68, 132], BF16, name=f"xnT{i}", tag=f"xnT{i}")
                   for i in (0, 1)]
            zd = [main.tile([128, HS * W], BF16, name=f"zd{i}", tag=f"zd{i}")
                  for i in (0, 1)]
            zc = [main.tile([128, HS, W], BF16, name=f"zc{i}", tag=f"zc{i}")
                  for i in (0, 1)]
            st_own = main.tile([128, 17, 4, 8], F32, name="st_own", tag="st_own")
            st_oth = main.tile([128, 16, 4, 8], F32, name="st_oth", tag="st_oth")
            mu_ow = main.tile([128, 17, 4], F32, name="mu_ow", tag="mu_ow")
            iv_ow = main.tile([128, 17, 4], F32, name="iv_ow", tag="iv_ow")
            mu_ot = main.tile([128, 16, 4], F32, name="mu_ot", tag="mu_ot")
            iv_ot = main.tile([128, 16, 4], F32, name="iv_ot", tag="iv_ot")
            winv = main.tile([128, 16, 4, 2], F32, name="winv", tag="winv")
            ownacc = main.tile([128, 2, NG], F32, name="ownacc", tag="ownacc")
            wdet_t = main.tile([128, 2, C], BF16, name="wdet_t", tag="wdet_t")
            wgeo_t = main.tile([128, 20, C], BF16, name="wgeo_t", tag="wgeo_t")
            wgimg_t = main.tile([128, 8, C], BF16, name="wgimg_t", tag="wgimg_t")
            wgate_t = main.tile([128, 4, C], BF16, name="wgate_t", tag="wgate_t")
            k1t = main.tile([128, 2, 9], F32, name="k1t", tag="k1t")
            k2t = main.tile([128, 2, 9], F32, name="k2t", tag="k2t")
            bnt = main.tile([128, 2, 2], F32, name="bnt", tag="bnt")
            lngt = main.tile([128, 2, 1], F32, name="lngt", tag="lngt")
            glst = main.tile([128, 2, 1], F32, name="glst", tag="glst")
            emt = main.tile([128, 2], F32, name="emt", tag="emt")
            ident = main.tile([128, 128], BF16, name="ident", tag="ident")
            ones_col = main.tile([128, 1], F32, name="ones_col", tag="ones_col")
            onesk1 = main.tile([1, 128], F32, name="onesk1", tag="onesk1")
            m_t = main.tile([128, 2], F32, name="m_t", tag="m_t")
            mr_t = main.tile([128, 8], F32, name="mr_t", tag="mr_t")
            gi_t = main.tile([128, 2, C], BF16, name="gi_t", tag="gi_t")
            pos_t = main.tile([128, 4], F32, name="pos_t", tag="pos_t")
            pb_t = main.tile([128, 2], F32, name="pb_t", tag="pb_t")
            eps_t = main.tile([128, 1], F32, name="eps_t", tag="eps_t")

            from concourse.masks import make_identity
            make_identity(nc, ident[:])
            nc.vector.memset(ones_col[:], 1.0)
            nc.vector.memset(onesk1[:], 1.0)
            nc.vector.memset(eps_t[:], LN_EPS)
            nc.sync.dma_start(out=wdet_t[:], in_=wdet.rearrange("a p c -> p a c"))
            nc.sync.dma_start(out=wgeo_t[:], in_=wgeo.rearrange("a p c -> p a c"))
            nc.sync.dma_start(out=wgimg_t[:], in_=wgimg.rearrange("a p c -> p a c"))
            nc.sync.dma_start(out=wgate_t[:], in_=wgate.rearrange("a p c -> p a c"))
            nc.sync.dma_start(out=k1t[:], in_=k1d.rearrange("a p c -> p a c"))
            nc.sync.dma_start(out=k2t[:], in_=k2d.rearrange("a p c -> p a c"))
            nc.sync.dma_start(out=bnt[:], in_=bnsb.rearrange("a p c -> p a c"))
            nc.sync.dma_start(out=lngt[:], in_=lng.rearrange("a p c -> p a c"))
            nc.sync.dma_start(out=glst[:], in_=glsd.rearrange("a p c -> p a c"))
            nc.sync.dma_start(out=emt[:], in_=emask[:])
            for i in (0, 1):
                nc.vector.memset(xnT[i][:], 0.0)

            # ================= phase A =================
            with tc.tile_pool(name="xio", bufs=3) as xio, \
                 tc.tile_pool(name="tmpa", bufs=2) as tmpa:
                for g in range(17):
                    xg = xio.tile([128, 4, 260], F32, name="xg", tag="xg")
                    nc.sync.dma_start(out=xg[:, :, 0:C],
                                      in_=x_slab[4 * g:4 * g + 4].rearrange("r p c -> p r c"))
                    for hp in range(4):
                        nc.vector.bn_stats(out=st_own[:, g, hp, 0:6],
                                           in_=xg[:, hp, 0:C])
                for g in range(16):
                    xg = xio.tile([128, 4, 260], F32, name="xg", tag="xg")
                    nc.sync.dma_start(out=xg[:, :, 0:C],
                                      in_=x_oth[4 * g:4 * g + 4].rearrange("r p c -> p r c"))
                    for hp in range(4):
                        nc.vector.bn_stats(out=st_oth[:, g, hp, 0:6],
                                           in_=xg[:, hp, 0:C])

                def finalize(st, mu_o, iv_o, ngr):
                    me, mo = st[:, :, :, 1:2], st[:, :, :, 4:5]
                    cve, cvo = st[:, :, :, 2:3], st[:, :, :, 5:6]
                    muv = mu_o[:].rearrange("p a (b o) -> p a b o", o=1)
                    ivv = iv_o[:].rearrange("p a (b o) -> p a b o", o=1)
                    d_t = tmpa.tile([128, ngr, 4, 1], F32, name="fin_d", tag="fin_d")
                    nc.vector.tensor_tensor(out=muv, in0=me, in1=mo, op=AL.add)
                    nc.vector.tensor_scalar(out=muv, in0=muv, scalar1=0.5,
                                            scalar2=None, op0=AL.mult)
                    nc.vector.tensor_tensor(out=d_t[:], in0=me, in1=mo,
                                            op=AL.subtract)
                    nc.vector.tensor_scalar(out=d_t[:], in0=d_t[:], scalar1=0.5,
                                            scalar2=None, op0=AL.mult)
                    nc.vector.tensor_tensor(out=d_t[:], in0=d_t[:], in1=d_t[:],
                                            op=AL.mult)
                    nc.vector.tensor_tensor(out=ivv, in0=cve, in1=cvo, op=AL.add)
                    nc.vector.scalar_tensor_tensor(out=ivv, in0=ivv, scalar=1.0 / C,
                                                   in1=d_t[:], op0=AL.mult,
                                                   op1=AL.add)
                    nc.scalar.activation(out=ivv, in_=ivv, func=AF.Sqrt,
                                         bias=eps_t[:, 0:1], scale=1.0)
                    nc.vector.reciprocal(out=ivv, in_=ivv)
                finalize(st_own, mu_ow, iv_ow, 17)
                finalize(st_oth, mu_ot, iv_ot, 16)
                nc.vector.tensor_copy(winv[:, :, :, 0:1],
                                      iv_ot[:].rearrange("p a (b o) -> p a b o", o=1))
                nc.vector.tensor_tensor(
                    out=winv[:, :, :, 1:2],
                    in0=mu_ot[:].rearrange("p a (b o) -> p a b o", o=1),
                    in1=iv_ot[:].rearrange("p a (b o) -> p a b o", o=1), op=AL.mult)

                # ---- A2: normalize + transpose + evac (+ own sums) ----
                with tc.tile_pool(name="ptr", bufs=2, space="PSUM") as ptr:
                    for g in range(17):
                        xg = xio.tile([128, 4, 260], F32, name="xg", tag="xg")
                        nc.sync.dma_start(out=xg[:, :, 0:C],
                                          in_=x_slab[4 * g:4 * g + 4].rearrange("r p c -> p r c"))
                        xn_tm = tmpa.tile([128, 4, C], BF16, name="xn_tm",
                                          tag="xn_tm")
                        for r in range(4):
                            nc.vector.tensor_scalar(
                                out=xn_tm[:, r, :], in0=xg[:, r, 0:C],
                                scalar1=mu_ow[:, g, r:r + 1], op0=AL.subtract,
                                scalar2=iv_ow[:, g, r:r + 1], op1=AL.mult)
                        for kc in (0, 1):
                            ps = ptr.tile([128, 4, 128], BF16, name="ps_tr",
                                          tag="ps_tr")
                            for r in range(4):
                                nc.tensor.transpose(
                                    ps[:, r, :],
                                    xn_tm[:, r, 128 * kc:128 * kc + 128], ident[:])
                            if g < 16:
                                nc.scalar.activation(
                                    out=xnT[kc][:, 4 * g + 2:4 * g + 6, 2:130],
                                    in_=ps[:], func=AF.Copy, scale=lngt[:, kc, 0:1],
                                    accum_out=ownacc[:, kc, g:g + 1])
                            else:
                                nc.scalar.activation(
                                    out=xnT[kc][:, 0:2, 2:130], in_=ps[:, 0:2, :],
                                    func=AF.Copy, scale=lngt[:, kc, 0:1])
                                nc.scalar.activation(
                                    out=xnT[kc][:, 66:68, 2:130], in_=ps[:, 2:4, :],
                                    func=AF.Copy, scale=lngt[:, kc, 0:1])

                # ---- other-half weighted sums ----
                with tc.tile_pool(name="pos", bufs=1, space="PSUM") as posp:
                    po0 = posp.tile([128, 1], F32, name="po0", tag="po0")
                    po1 = posp.tile([128, 1], F32, name="po1", tag="po1")
                    poo = posp.tile([1, 2], F32, name="poo", tag="poo")
                    po_mc = (po0, po1)
                    n_mm = 16 * 4
                    i_mm = 0
                    for g in range(16):
                        xg = xio.tile([128, 4, 260], F32, name="xg", tag="xg")
                        nc.sync.dma_start(out=xg[:, :, 0:C],
                                          in_=x_oth[4 * g:4 * g + 4].rearrange("r p c -> p r c"))
                        for r in range(4):
                            st_f, sp_f = (i_mm == 0), (i_mm == n_mm - 1)
                            for mc in (0, 1):
                                nc.tensor.matmul(po_mc[mc][:],
                                                 xg[:, r, 128 * mc:128 * mc + 128],
                                                 winv[:, g, r, 0:1], start=st_f,
                                                 stop=sp_f)
                            nc.tensor.matmul(poo[:], ones_col[:],
                                             winv[:, g, r, :], start=st_f, stop=sp_f)
                            i_mm += 1
                    nc.scalar.copy(pos_t[:, 0:1], po0[:])
                    nc.scalar.copy(pos_t[:, 1:2], po1[:])
                    nc.scalar.copy(pos_t[0:1, 2:4], poo[:])
                with tc.tile_pool(name="pbb", bufs=1, space="PSUM") as pbb:
                    pb = pbb.tile([128, 2], F32, name="pb", tag="pb")
                    nc.tensor.matmul(pb[:], onesk1[:], pos_t[0:1, 2:4], start=True,
                                     stop=True)
                    nc.scalar.copy(pb_t[:], pb[:])

                # ---- image mean m and G_img ----
                for kc in (0, 1):
                    ot = tmpa.tile([128, 1], F32, name="m_ot", tag="m_ot")
                    nc.vector.tensor_reduce(out=ot[:], in_=ownacc[:, kc, :],
                                            axis=AX.X, op=AL.add)
                    t2 = tmpa.tile([128, 1], F32, name="m_t2", tag="m_t2")
                    nc.vector.tensor_tensor(out=t2[:], in0=pos_t[:, kc:kc + 1],
                                            in1=pb_t[:, 1:2], op=AL.subtract)
                    nc.vector.tensor_tensor(out=t2[:], in0=t2[:],
                                            in1=lngt[:, kc, :], op=AL.mult)
                    nc.vector.tensor_tensor(out=t2[:], in0=t2[:], in1=ot[:],
                                            op=AL.add)
                    nc.vector.tensor_scalar(out=m_t[:, kc:kc + 1], in0=t2[:],
                                            scalar1=1.0 / (H * W), scalar2=None,
                                            op0=AL.mult)
                for j, s in enumerate((1, 2, -1, -2)):
                    _mroll_dma(nc, mr_t, m_t, s, 2 * j)
                for kc in (0, 1):
                    nc.vector.tensor_scalar(out=gi_t[:, kc, :],
                                            in0=wgimg_t[:, 0 + kc, :],
                                            scalar1=mr_t[:, kc:kc + 1],
                                            scalar2=None, op0=AL.mult)
                    for j in (1, 2, 3):
                        nc.vector.scalar_tensor_tensor(
                            out=gi_t[:, kc, :], in0=wgimg_t[:, 2 * j + kc, :],
                            scalar=mr_t[:, 2 * j + kc:2 * j + kc + 1],
                            in1=gi_t[:, kc, :], op0=AL.mult, op1=AL.add)

            # ================= z_det =================
            with tc.tile_pool(name="pzd", bufs=2, space="PSUM") as pzd:
                for g in range(NG):
                    for mc in (0, 1):
                        ps = pzd.tile([128, 512], F32, name="ps_zd", tag="ps_zd")
                        for kc in (0, 1):
                            nc.tensor.matmul(
                                ps[:], wdet_t[:, kc, 128 * mc:128 * mc + 128],
                                xnT[kc][:, 4 * g + 2:4 * g + 6, 2:130],
                                start=(kc == 0), stop=(kc == 1))
                        nc.scalar.copy(zd[mc][:, 512 * g:512 * g + 512], ps[:])

            # ================= convs =================
            taps = [(dh, dw) for dh in (-1, 0, 1) for dw in (-1, 0, 1)]
            with tc.tile_pool(name="z1p", bufs=1) as z1p:
                z1 = [z1p.tile([128, WP2, WW2], BF16, name=f"z1_{i}", tag=f"z1_{i}")
                      for i in (0, 1)]
                for ci in (0, 1):
                    for t, (dh, dw) in enumerate(taps):
                        src = xnT[ci][:, dh + 1:dh + 1 + WP2, dw + 1:dw + 1 + WW2]
                        if t == 0:
                            nc.vector.tensor_scalar(out=z1[ci][:], in0=src,
                                                    scalar1=k1t[:, ci, t:t + 1],
                                                    scalar2=None, op0=AL.mult)
                        else:
                            nc.vector.scalar_tensor_tensor(
                                out=z1[ci][:], in0=src, scalar=k1t[:, ci, t:t + 1],
                                in1=z1[ci][:], op0=AL.mult, op1=AL.add)
                    nc.vector.memset(z1[ci][:, :, 0:1], 0.0)
                    nc.vector.memset(z1[ci][:, :, 129:130], 0.0)
                    nc.vector.tensor_scalar(out=z1[ci][:, 0, :],
                                            in0=z1[ci][:, 0, :],
                                            scalar1=emt[:, 0:1], scalar2=None,
                                            op0=AL.mult)
                    nc.vector.tensor_scalar(out=z1[ci][:, 65, :],
                                            in0=z1[ci][:, 65, :],
                                            scalar1=emt[:, 1:2], scalar2=None,
                                            op0=AL.mult)
                    for t, (dh, dw) in enumerate(taps):
                        src = z1[ci][:, dh + 1:dh + 1 + HS, dw + 1:dw + 1 + W]
                        if t == 0:
                            nc.vector.tensor_scalar(out=zc[ci][:], in0=src,
                                                    scalar1=k2t[:, ci, t:t + 1],
                                                    scalar2=None, op0=AL.mult)
                        else:
                            nc.vector.scalar_tensor_tensor(
                                out=zc[ci][:], in0=src, scalar=k2t[:, ci, t:t + 1],
                                in1=zc[ci][:], op0=AL.mult, op1=AL.add)
                    flat = zc[ci][:].rearrange("p h w -> p (h w)")
                    nc.scalar.activation(out=flat, in_=flat, func=AF.Silu,
                                         bias=bnt[:, ci, 1:2], scale=bnt[:, ci, 0:1])

            # ================= phase B =================
            with tc.tile_pool(name="ftp", bufs=2) as ftp, \
                 tc.tile_pool(name="rlp", bufs=2) as rlp, \
                 tc.tile_pool(name="gtp", bufs=2) as gtp, \
                 tc.tile_pool(name="xio2", bufs=2) as xio2, \
                 tc.tile_pool(name="pgf", bufs=1, space="PSUM") as pgf, \
                 tc.tile_pool(name="pgt", bufs=1, space="PSUM") as pgt, \
                 tc.tile_pool(name="pht", bufs=1, space="PSUM") as pht:
                for sg in range(NSG):
                    lo, hi = SGTOK * sg, SGTOK * (sg + 1)
                    bsg = gtp.tile([128, 2, SGTOK], BF16, name="bsg", tag="bsg")
                    for ci in (0, 1):
                        zcf = zc[ci][:].rearrange("p h w -> p (h w)")
                        nc.vector.tensor_tensor(out=bsg[:, ci, :],
                                                in0=zcf[:, lo:hi],
                                                in1=zd[ci][:, lo:hi],
                                                op=AL.subtract)
                    psg = {}
                    for gl in range(SG):
                        for mc in (0, 1):
                            psg[(gl, mc)] = pgf.tile(
                                [128, 512], F32, name=f"psg{gl}{mc}",
                                tag=f"psg{gl}{mc}")
                    blk = 0
                    NBLK = 22

                    def mm4(lhs_idx, ft):
                        nonlocal blk
                        for kc in (0, 1):
                            for gl in range(SG):
                                rhs = ft[:, kc, 512 * gl:512 * gl + 512]
                                for mc in (0, 1):
                                    nc.tensor.matmul(
                                        psg[(gl, mc)],
                                        wgeo_t[:, lhs_idx + kc,
                                               128 * mc:128 * mc + 128],
                                        rhs, start=(blk == 0),
                                        stop=(blk == NBLK - 1))
                            blk += 1

                    bi = 0
                    for s in SHIFTS_L:
                        zr = rlp.tile([128, 2, SGTOK], BF16, name="zr", tag="zr")
                        br = rlp.tile([128, 2, SGTOK], BF16, name="br", tag="br")
                        _roll_dma(nc, zr, (zd[0][:], zd[1][:]), s, (lo, hi))
                        _roll_dma(nc, br, (bsg[:, 0, :], bsg[:, 1, :]), s,
                                  (0, SGTOK))
                        ftR = ftp.tile([128, 2, SGTOK], BF16, name="ftR", tag="ftR")
                        ftT = ftp.tile([128, 2, SGTOK], BF16, name="ftT", tag="ftT")
                        for ci in (0, 1):
                            nc.vector.tensor_tensor(out=ftR[:, ci, :],
                                                    in0=zd[ci][:, lo:hi],
                                                    in1=br[:, ci, :], op=AL.mult)
                            nc.vector.tensor_tensor(out=ftT[:, ci, :],
                                                    in0=zr[:, ci, :],
                                                    in1=bsg[:, ci, :], op=AL.mult)
                        mm4(2 * bi, ftR); bi += 1
                        mm4(2 * bi, ftT); bi += 1
                        if s in SHIFTS_G:
                            ftS = ftp.tile([128, 2, SGTOK], BF16, name="ftS",
                                           tag="ftS")
                            for ci in (0, 1):
                                nc.vector.tensor_tensor(out=ftS[:, ci, :],
                                                        in0=zr[:, ci, :],
                                                        in1=zd[ci][:, lo:hi],
                                                        op=AL.mult)
                            mm4(2 * bi, ftS); bi += 1
                    # G_img block (rhs = zd window)
                    for kc in (0, 1):
                        for gl in range(SG):
                            glo = lo + 512 * gl
                            for mc in (0, 1):
                                nc.tensor.matmul(
                                    psg[(gl, mc)],
                                    gi_t[:, kc, 128 * mc:128 * mc + 128],
                                    zd[kc][:, glo:glo + 512],
                                    start=False, stop=(blk == NBLK - 1))
                        blk += 1

                    for gl in range(SG):
                        g = SG * sg + gl
                        gp = gtp.tile([128, 2, 512], BF16, name="gp", tag="gp")
                        for mc in (0, 1):
                            nc.scalar.activation(out=gp[:, mc, :],
                                                 in_=psg[(gl, mc)], func=AF.Copy,
                                                 scale=glst[:, mc, 0:1])
                        pg = pgt.tile([128, 2, 512], F32, name="pg", tag="pg")
                        for mc in (0, 1):
                            for kc in (0, 1):
                                nc.tensor.matmul(
                                    pg[:, mc, :],
                                    wgate_t[:, kc, 128 * mc:128 * mc + 128],
                                    xnT[kc][:, 4 * g + 2:4 * g + 6, 2:130],
                                    start=(kc == 0), stop=False)
                            for kc in (0, 1):
                                nc.tensor.matmul(
                                    pg[:, mc, :],
                                    wgate_t[:, 2 + kc, 128 * mc:128 * mc + 128],
                                    gp[:, kc, :], start=False, stop=(kc == 1))
                        al = gtp.tile([128, 2, 512], BF16, name="al", tag="al")
                        sgm = gtp.tile([128, 2, 512], BF16, name="sgm", tag="sgm")
                        hh = gtp.tile([128, 2, 512], BF16, name="hh", tag="hh")
                        for mc in (0, 1):
                            nc.scalar.activation(out=al[:, mc, :], in_=pg[:, mc, :],
                                                 func=AF.Sigmoid)
                            nc.scalar.activation(
                                out=sgm[:, mc, :],
                                in_=xnT[mc][:, 4 * g + 2:4 * g + 6, 2:130],
                                func=AF.Sigmoid)
                            nc.vector.tensor_tensor(out=al[:, mc, :],
                                                    in0=al[:, mc, :],
                                                    in1=gp[:, mc, :], op=AL.mult)
                            nc.vector.tensor_tensor(
                                out=sgm[:, mc, :], in0=sgm[:, mc, :],
                                in1=xnT[mc][:, 4 * g + 2:4 * g + 6, 2:130],
                                op=AL.mult)
                            nc.vector.scalar_tensor_tensor(
                                out=hh[:, mc, :], in0=sgm[:, mc, :],
                                scalar=glst[:, mc, 0:1], in1=al[:, mc, :],
                                op0=AL.mult, op1=AL.add)
                        ph = pht.tile([128, 4, C], BF16, name="ph", tag="ph")
                        for r in range(4):
                            for mc in (0, 1):
                                nc.tensor.transpose(
                                    ph[:, r, 128 * mc:128 * mc + 128],
                                    hh[:, mc, 128 * r:128 * r + 128], ident[:])
                        xf = xio2.tile([128, 4, 260], F32, name="xf", tag="xf")
                        nc.sync.dma_start(
                            out=xf[:, :, 0:C],
                            in_=x_slab[4 * g:4 * g + 4].rearrange("r p c -> p r c"))
                        of = xio2.tile([128, 4, C], F32, name="of", tag="of")
                        nc.vector.tensor_tensor(out=of[:], in0=xf[:, :, 0:C],
                                                in1=ph[:], op=AL.add)
                        nc.sync.dma_start(
                            out=y[4 * g:4 * g + 4].rearrange("r p c -> p r c"),
                            in_=of[:])
    nc.compile()
    return nc


# ---------------------------------------------------------------------------
# host side
# ---------------------------------------------------------------------------
_PROG = {}


def _get_program(n_cores=N_CORES):
    if n_cores not in _PROG:
        _PROG[n_cores] = build_program(n_cores)
    return _PROG[n_cores]


def _prep_weights(inputs):
    w_det = np.asarray(inputs["w_det"], np.float32)
    dw1 = np.asarray(inputs["dw1"], np.float32)
    dw2 = np.asarray(inputs["dw2"], np.float32)
    bn_gamma = np.asarray(inputs["bn_gamma"], np.float32)
    bn_mean = np.asarray(inputs["bn_mean"], np.float32)
    bn_var = np.asarray(inputs["bn_var"], np.float32)
    wl = np.asarray(inputs["w_geo_local"], np.float32)
    wg = np.asarray(inputs["w_geo_global"], np.float32)
    w_gate = np.asarray(inputs["w_gate"], np.float32)
    gls = np.asarray(inputs["gamma_ls"], np.float32)
    lng = np.asarray(inputs["ln_gamma"], np.float32)

    Wl = wl.reshape(4, 2, C, C)
    Wu_l, Wv_l = Wl[:, 0] + Wl[:, 1], Wl[:, 0] - Wl[:, 1]
    Wg2 = wg.reshape(2, 2, C, C)
    Wu_g, Wv_g = Wg2[:, 0] + Wg2[:, 1], Wg2[:, 0] - Wg2[:, 1]
    blocks = {}
    for i, s in enumerate(SHIFTS_L):
        blocks[("R", s)] = Wu_l[i]
        blocks[("T", s)] = Wv_l[i]
    for i, s in enumerate(SHIFTS_G):
        blocks[("S", s)] = -2.0 * Wg2[i, 0]
    wgeo = np.stack([blocks[b][kc * 128:(kc + 1) * 128]
                     for b in BLOCKS for kc in (0, 1)]).astype(BFNP)
    wvr = [np.roll(Wv_g[i], -(i + 1), axis=0) for i in range(2)]
    gim = [Wu_g[0], Wu_g[1], wvr[0], wvr[1]]
    wgimg = np.stack([gim[j][kc * 128:(kc + 1) * 128]
                      for j in range(4) for kc in (0, 1)]).astype(BFNP)
    gls_safe = np.where(np.abs(gls) < 1e-30, 1.0, gls)
    w_gate_adj = w_gate.copy()
    w_gate_adj[C:] = w_gate_adj[C:] / gls_safe[:, None]
    wgate = np.stack([w_gate_adj[kc * 128:(kc + 1) * 128]
                      for kc in range(4)]).astype(BFNP)
    wdet = np.stack([w_det[kc * 128:(kc + 1) * 128] for kc in (0, 1)]).astype(BFNP)

    def ktap(dw):
        # [2, 128, 9]: per chunk, per channel, taps in (dh, dw) row-major order
        k = dw[:, :, 0, :]                      # [3, 3, C]
        k = k.reshape(9, C).T                  # [C, 9]
        return np.stack([k[:128], k[128:]]).astype(np.float32)

    bn_s = bn_gamma / np.sqrt(bn_var + BN_EPS)
    bn_b = -bn_mean * bn_s
    bnsb = np.stack([np.stack([bn_s[kc * 128:(kc + 1) * 128],
                               bn_b[kc * 128:(kc + 1) * 128]], axis=1)
                     for kc in (0, 1)]).astype(np.float32)
    lngs = np.stack([lng[kc * 128:(kc + 1) * 128, None] for kc in (0, 1)])
    glss = np.stack([gls[kc * 128:(kc + 1) * 128, None] for kc in (0, 1)])
    return dict(wdet=wdet, wgeo=wgeo, wgimg=wgimg, wgate=wgate,
                k1=ktap(dw1), k2=ktap(dw2), bnsb=bnsb,
                lng=lngs.astype(np.float32), gls=glss.astype(np.float32))


def kernel(**inputs):
    x = np.asarray(inputs["x"], np.float32)
    shared = _prep_weights(inputs)
    nc = _get_program(N_CORES)
    in_maps = []
    for k in range(N_CORES):
        b, half = k // 2, k % 2
        h0 = HS * half
        slab = np.zeros((68, W, C), np.float32)
        slab[0:HS] = x[b, h0:h0 + HS]
        if h0 > 0:
            slab[64] = x[b, h0 - 2]
            slab[65] = x[b, h0 - 1]
        if h0 + HS < H:
            slab[66] = x[b, h0 + HS]
            slab[67] = x[b, h0 + HS + 1]
        oth0 = HS - h0
        x_oth = np.ascontiguousarray(x[b, oth0:oth0 + HS])
        emask = np.zeros((128, 2), np.float32)
        emask[:, 0] = 0.0 if h0 == 0 else 1.0
        emask[:, 1] = 0.0 if h0 + HS == H else 1.0
        im = dict(x_slab=slab, x_oth=x_oth, emask=emask)
        im.update({k2: v.copy() for k2, v in shared.items()})
        in_maps.append(im)
    res = bass_utils.run_bass_kernel_spmd(nc, in_maps, core_ids=list(range(N_CORES)))
    out = np.empty((B, H, W, C), np.float32)
    for k in range(N_CORES):
        b, half = k // 2, k % 2
        out[b, HS * half:HS * half + HS] = res.results[k]["y"]
    return out
